# revision 12
# baseline (speedup 1.0000x reference)
"""Causal self-attention with post-softmax decay mask — Trainium2 Bass kernel.

Problem shapes (hardcoded): B=2, T=2048, C=1024, H=16 heads, head_dim=64.
Sharding: 8 cores = (batch b = core//4) x (head group g = core%4, 4 heads each).
Each core computes QKV projection for its 4 heads from x[b], causal
flash-style attention with the decay mask, and a partial output projection
(its heads' 256 features x W_proj rows). Host sums the 4 partials per batch.

Per-core kernel (all matmuls bf16, fp32 PSUM accumulation):
  phase 1: qkv^T strips (Q^T/K^T, [64feat x T] pairs packed into 128
           partitions) + V natural [T x 256] from xT and the weight slice.
  phase 2: per q-tile (128 rows) per head: S = Q K^T (row-packed matmul
           pairs), causal mask via -1e30 add on the diagonal 128x128 chunk,
           exp via ScalarE (scale=1/8) with accum_out giving row sums Z,
           Pd = (P * (1/Z)) * decay in one scalar_tensor_tensor, transpose
           Pd 128x128 blocks with DMA XBAR, PV matmuls (col-packed head
           pairs) accumulating y^T, then projection + output DMA.
"""

import math
import sys

sys.path.insert(0, "/opt/trn_rl_repo")

import numpy as np
import ml_dtypes

B, T, C = 2, 2048, 1024
N_HEAD = 16
HD = 64
HEADS_PER_CORE = 4
N_CORES = 8
NQ = T // 128  # 16 q-tiles

BF16 = ml_dtypes.bfloat16


def _decay_matrix_np(n):
    """tril decay matrix, faithful to reference.decay_weight_matrix (fp32)."""
    dl = 2048 - 16 + 1
    nums = np.linspace(0.0, 1.0, dl, dtype=np.float64)
    decay_values = 1.0 - np.power(nums, 1.0 / np.e)
    decay_values = np.concatenate([np.ones(15), decay_values])[:n]
    idx = np.arange(n)[:, None] - np.arange(n)[None, :]
    mat = decay_values[np.clip(idx, 0, n - 1)]
    return np.where(idx >= 0, mat, 0.0).astype(np.float32)


def build_nc(T_=T):
    import concourse.bass as bass
    import concourse.bacc as bacc
    import concourse.mybir as mybir
    import concourse.tile as tile

    fp32 = mybir.dt.float32
    bf16 = mybir.dt.bfloat16
    Alu = mybir.AluOpType
    Act = mybir.ActivationFunctionType

    NQ_ = T_ // 128
    nc = bacc.Bacc("TRN2")

    def copy_dve(out, in_):
        nc.vector.tensor_copy(out=out, in_=in_)

    def copy_act(out, in_):
        nc.scalar.copy(out=out, in_=in_)

    xT = nc.declare_dram_parameter("xT", [C, T_], bf16, isOutput=False)
    # wqkv columns: [q01(128) q23(128) k01(128) k23(128) v0..v3(256)]
    wqkv = nc.declare_dram_parameter("wqkv", [C, 768], bf16, isOutput=False)
    # wp rows: h0 feats(64), h1, h2, h3
    wp = nc.declare_dram_parameter("wp", [256, C], bf16, isOutput=False)
    decay = nc.declare_dram_parameter("decay", [T_, T_], bf16, isOutput=False)
    maskneg = nc.declare_dram_parameter("maskneg", [128, 128], fp32, isOutput=False)
    out = nc.declare_dram_parameter("out", [T_, C], fp32, isOutput=True)

    with tile.TileContext(nc) as tc:
        with (
            tc.tile_pool(name="const", bufs=1) as const_pool,
            tc.tile_pool(name="qkvout", bufs=1) as qkv_pool,
            tc.tile_pool(name="dec", bufs=2) as dec_pool,
            tc.tile_pool(name="p", bufs=3) as p_pool,
            tc.tile_pool(name="pdt", bufs=3) as pdt_pool,
            tc.tile_pool(name="z", bufs=8) as z_pool,
            tc.tile_pool(name="outs", bufs=2) as out_pool,
            tc.tile_pool(name="ps_s", bufs=3, space="PSUM") as ps_s,
            tc.tile_pool(name="ps_y", bufs=2, space="PSUM") as ps_y_pool,
        ):
            # ---- load constants / inputs ----
            xt_sb = const_pool.tile([128, 8, T_], bf16)
            nc.sync.dma_start(out=xt_sb, in_=xT.rearrange("(kc p) t -> p kc t", p=128))
            wqkv_sb = const_pool.tile([128, 8, 768], bf16)
            nc.sync.dma_start(
                out=wqkv_sb, in_=wqkv.rearrange("(kc p) m -> p kc m", p=128)
            )
            wp_sb = const_pool.tile([128, 2, C], bf16)
            nc.sync.dma_start(out=wp_sb, in_=wp.rearrange("(pr p) n -> p pr n", p=128))
            mask_dma = const_pool.tile([128, 128], fp32)
            nc.sync.dma_start(out=mask_dma, in_=maskneg[:, :])
            # DVE-owned copy: diagonal mask adds then depend on DVE program
            # order instead of a DMA semaphore (the TT struct encodes only
            # one sync wait).
            mask_sb = const_pool.tile([128, 128], fp32)
            nc.vector.tensor_copy(out=mask_sb, in_=mask_dma)

            qt_sb = qkv_pool.tile([128, 2, T_], bf16)  # [feat(2x64), pair, q]
            kt_sb = qkv_pool.tile([128, 2, T_], bf16)
            v_sb = qkv_pool.tile([128, T_ // 128, 256], bf16)  # [k-rows, kc, 4 heads]
            yt_sb = qkv_pool.tile([128, 2, T_], bf16)  # y^T strips per pair

            # ---- phase 1: QKV projections ----
            # Q^T / K^T strips: lhsT = w chunk [128C x 128 (2 heads)], rhs = xT
            for which, dst in ((0, qt_sb), (1, kt_sb)):
                for pair in range(2):
                    wcol = 256 * which + 128 * pair
                    for nqc in range((T_ + 511) // 512):  # T in chunks of 512
                        cw = min(512, T_ - 512 * nqc)
                        ps_full = ps_s.tile([128, 1024], fp32, tag="s")
                        ps = ps_full[:, 0:512]
                        for kc in range(8):
                            nc.tensor.matmul(
                                ps[:, 0:cw],
                                lhsT=wqkv_sb[:, kc, wcol : wcol + 128],
                                rhs=xt_sb[:, kc, 512 * nqc : 512 * nqc + cw],
                                start=(kc == 0),
                                stop=(kc == 7),
                            )
                        cp = copy_dve if (nqc % 2 == 0) else copy_act
                        cp(dst[:, pair, 512 * nqc : 512 * nqc + cw], ps[:, 0:cw])
            # V natural: lhsT = xT chunk (stationary), rhs = wv cols
            for tc16 in range(T_ // 128):
                ps_full = ps_s.tile([128, 1024], fp32, tag="s")
                ps = ps_full[:, 0:512]
                for kc in range(8):
                    nc.tensor.matmul(
                        ps[:, 0:256],
                        lhsT=xt_sb[:, kc, 128 * tc16 : 128 * tc16 + 128],
                        rhs=wqkv_sb[:, kc, 512:768],
                        start=(kc == 0),
                        stop=(kc == 7),
                    )
                cp = copy_dve if (tc16 % 2 == 0) else copy_act
                cp(v_sb[:, tc16, :], ps[:, 0:256])

            # ---- phase 2: attention per q-tile ----
            for tq in range(NQ_):
                L = 128 * (tq + 1)
                d_t = dec_pool.tile([128, T_], bf16, tag="dec")
                nc.sync.dma_start(
                    out=d_t[:, 0:L], in_=decay[128 * tq : 128 * tq + 128, 0:L]
                )
                for pair in range(2):
                    ps_y = ps_y_pool.tile([128, 128], fp32)
                    for hin in range(2):
                        head = 2 * pair + hin
                        prow = 64 * hin
                        p_t = p_pool.tile([128, T_], bf16, tag="p")
                        npieces = (L + 1023) // 1024
                        zparts = z_pool.tile([128, 2], fp32, tag="zp")
                        for piece in range(npieces):
                            p0 = 1024 * piece
                            pl = min(1024, L - p0)
                            ps = ps_s.tile([128, 1024], fp32, tag="s")
                            for sc in range((pl + 511) // 512):
                                scl = min(512, pl - 512 * sc)
                                k0 = p0 + 512 * sc
                                nc.tensor.matmul(
                                    ps[:, 512 * sc : 512 * sc + scl],
                                    lhsT=qt_sb[
                                        prow : prow + 64,
                                        pair,
                                        128 * tq : 128 * tq + 128,
                                    ],
                                    rhs=kt_sb[prow : prow + 64, pair, k0 : k0 + scl],
                                    start=True,
                                    stop=True,
                                    tile_position=(prow, 0),
                                )
                            if p0 + pl == L:  # diagonal chunk lives here
                                off = (L - 128) - p0
                                # stt instead of tensor_tensor: the TT ISA
                                # struct only encodes one sync wait and
                                # walrus refuses multi-wait TTs.
                                nc.vector.scalar_tensor_tensor(
                                    out=ps[:, off : off + 128],
                                    in0=ps[:, off : off + 128],
                                    scalar=1.0,
                                    in1=mask_sb,
                                    op0=Alu.mult,
                                    op1=Alu.add,
                                )
                            nc.scalar.activation(
                                out=p_t[:, p0 : p0 + pl],
                                in_=ps[:, 0:pl],
                                func=Act.Exp,
                                scale=0.125,
                                accum_out=zparts[:, piece : piece + 1],
                            )
                        z = z_pool.tile([128, 1], fp32, tag="z")
                        nc.vector.reduce_sum(
                            out=z, in_=zparts[:, 0:npieces], axis=mybir.AxisListType.X
                        )
                        rz = z_pool.tile([128, 1], fp32, tag="rz")
                        nc.vector.reciprocal(out=rz, in_=z)
                        nc.vector.tensor_scalar(
                            out=p_t[:, 0:L],
                            in0=p_t[:, 0:L],
                            scalar1=rz,
                            scalar2=None,
                            op0=Alu.mult,
                        )
                        nc.gpsimd.tensor_tensor(
                            out=p_t[:, 0:L],
                            in0=p_t[:, 0:L],
                            in1=d_t[:, 0:L],
                            op=Alu.mult,
                        )
                        pdt = pdt_pool.tile([128, NQ_, 128], bf16, tag="pdt")
                        # one XBAR-transpose instruction for all (tq+1)
                        # 128x128 blocks: out[:, b, :] = in[:, 128b:128b+128].T
                        nc.sync.dma_start_transpose(
                            out=pdt[:, 0 : tq + 1, :],
                            in_=p_t[:, 0:L],
                        )
                        for kc in range(tq + 1):
                            nc.tensor.matmul(
                                ps_y[prow : prow + 64, :],
                                lhsT=v_sb[:, kc, 64 * head : 64 * head + 64],
                                rhs=pdt[:, kc, :],
                                start=(kc == 0),
                                stop=(kc == tq),
                                tile_position=(0, prow),
                            )
                    cp = copy_dve if (pair == 0) else copy_act
                    cp(yt_sb[:, pair, 128 * tq : 128 * tq + 128], ps_y)
                # ---- projection for this q-tile ----
                o_t = out_pool.tile([128, C], fp32, tag="o")
                for nh in range(2):
                    ps_full = ps_s.tile([128, 1024], fp32, tag="s")
                    ps = ps_full[:, 0:512]
                    for pair in range(2):
                        nc.tensor.matmul(
                            ps,
                            lhsT=yt_sb[:, pair, 128 * tq : 128 * tq + 128],
                            rhs=wp_sb[:, pair, 512 * nh : 512 * nh + 512],
                            start=(pair == 0),
                            stop=(pair == 1),
                        )
                    cp = copy_dve if (nh == 0) else copy_act
                    cp(o_t[:, 512 * nh : 512 * nh + 512], ps)
                nc.sync.dma_start(
                    out=out[128 * tq : 128 * tq + 128, :], in_=o_t
                )

    nc.compile()
    return nc


def make_in_maps(x, W_attn, W_proj, T_=T):
    """Host-side sharding: per-core input dicts."""
    x = np.asarray(x, dtype=np.float32)[:, :T_, :]
    W_attn = np.asarray(W_attn, dtype=np.float32)
    W_proj = np.asarray(W_proj, dtype=np.float32)

    decay = _decay_matrix_np(T_).astype(BF16)
    mask = np.where(
        np.arange(128)[None, :] <= np.arange(128)[:, None], 0.0, -1e30
    ).astype(np.float32)

    in_maps = []
    for core in range(N_CORES):
        b = core // 4
        g = core % 4
        h0 = HEADS_PER_CORE * g  # first head of this core within the batch
        xT_c = np.ascontiguousarray(x[b].T).astype(BF16)  # [C, T]
        cols = []
        for which in range(2):  # q, k
            base = 1024 * which
            for pair in range(2):
                h = h0 + 2 * pair
                cols.append(W_attn[:, base + 64 * h : base + 64 * (h + 2)])
        cols.append(W_attn[:, 2048 + 64 * h0 : 2048 + 64 * (h0 + 4)])  # v
        wqkv_c = np.concatenate(cols, axis=1).astype(BF16)  # [C, 768]
        wp_c = W_proj[64 * h0 : 64 * (h0 + 4), :].astype(BF16)  # [256, C]
        in_maps.append(
            {
                "xT": xT_c,
                "wqkv": wqkv_c,
                "wp": wp_c,
                "decay": decay,
                "maskneg": mask,
            }
        )
    return in_maps


def kernel(x, W_attn, W_proj):
    from concourse.bass_utils import run_bass_kernel_spmd

    in_maps = make_in_maps(x, W_attn, W_proj)
    nc = build_nc()
    res = run_bass_kernel_spmd(nc, in_maps, core_ids=list(range(N_CORES)))
    outs = [np.asarray(r["out"], dtype=np.float32) for r in res.results]
    full = np.zeros((B, T, C), dtype=np.float32)
    for core in range(N_CORES):
        full[core // 4] += outs[core]
    return full
